# revision 23
# baseline (speedup 1.0000x reference)
"""Trainium2 Bass kernel for nn_LinearCoeffGNN: coeffs = U @ Vp^T pipeline.

Exact factorization of the reference:  coeffs[b] = F0e @ N_ext @ F0e^T

  F0e = [qv_0 qb_0 .. qv_7 qb_7 | 1 | x]  (P x 18, bf16) where
  qv_h(x), qb_h(x) are scalar C1 functions of x (the Linear(1,hid) layers
  make everything rank-1 in x).  They are evaluated as a 128-knot linear
  spline: ONE Relu activation rfeat[j,p] = relu(x_p - theta_j) plus a
  [128,16] matmul; the const/linear spline terms fold into N_ext via
  G rows 16/17 (fit max err 8e-4 on range 31).

  softmax stats: s(A) = num/den are moment generating functions of x
  (entire in A), so only den at KN chebyshev nodes is needed; all four
  batches' stats run as ONE [128,P] Exp + ONE reduce + ONE Ln + ONE
  block-diagonal Dmat matmul in the prologue (derivative of the
  chebyshev interpolant), then per-head sums S1,S2 via [32,49] matmuls.

  N_ext = G T' Mp T' G^T built as TG^T (Mp TG), TG = T'G^T from two
  Identity(scale) activations + 6 tiny accumulating matmuls.

  The batch loop only uses Relu/Identity activations (one act-table
  set: no table reloads), all consts arrive in one packed DMA, and
  batch b+1's front is software-pipelined over batch b's final block.
Sharding: data-parallel over batch B=32 -> 4 batches per core on 8 cores.
"""
import numpy as np
import ml_dtypes

import concourse.bacc as bacc
import concourse.bass as bass
import concourse.mybir as mybir
import concourse.tile as tile
from concourse import bass_utils

B, P = 32, 1024
HID, H, D = 512, 8, 64
MEM, RANK = 64, 64
NCORES = 8
BPC = B // NCORES  # batches per core
KN = 32            # chebyshev nodes for the softmax-stats interpolation
MK = 128           # spline knots for qv/qb evaluation

F32 = mybir.dt.float32
BF16 = mybir.dt.bfloat16
AF = mybir.ActivationFunctionType
ALU = mybir.AluOpType

_CACHE = {}
TRACE = False

# packed const layout: columns of a [128, NCC] f32 tensor
_C_NTH = 0      # [128,1] -theta
_C_CH4 = 1      # [128,1] chebyshev nodes tiled x4
_C_DT4 = 2      # [128,128] block-diag Dmat^T x4
_C_RA4 = 130    # [128,49] RA tiled x4
_C_RB4 = 179    # [128,49] RB tiled x4
_C_MAG = 228    # [17,18] mA G^T
_C_MBG = 246    # [17,18] mB G^T
_C_CTG = 264    # [17,18] cT G^T
_C_MT = 282     # [17,17] Mp^T
_C_CA = 299     # [128,8] coefa bf16 viewed as f32
NCC = 307


def _build():
    nc = bacc.Bacc("TRN2", target_bir_lowering=False, debug=False,
                   num_devices=NCORES)
    xs = nc.dram_tensor("xs", [BPC, P], F32, kind="ExternalInput").ap()
    cpack = nc.dram_tensor("cpack", [128, NCC], F32, kind="ExternalInput").ap()
    coefa = nc.dram_tensor("coefa", [MK, 16], BF16, kind="ExternalInput").ap()
    out = nc.dram_tensor("out", [BPC, P, P], F32, kind="ExternalOutput").ap()

    with tile.TileContext(nc) as tc:
        with tc.tile_pool(name="consts", bufs=1) as cp, \
             tc.tile_pool(name="work", bufs=2) as wp, \
             tc.tile_pool(name="stage", bufs=2) as sp, \
             tc.tile_pool(name="ps_big", bufs=1, space="PSUM") as psa, \
             tc.tile_pool(name="ps_small", bufs=1, space="PSUM") as pss, \
             tc.tile_pool(name="ps_c0", bufs=1, space="PSUM") as pc0, \
             tc.tile_pool(name="ps_c1", bufs=1, space="PSUM") as pc1, \
             tc.tile_pool(name="ps_c2", bufs=1, space="PSUM") as pc2, \
             tc.tile_pool(name="ps_c3", bufs=1, space="PSUM") as pc3, \
             tc.tile_pool(name="ps_c4", bufs=1, space="PSUM") as pc4:
            ccp = [pc0, pc1, pc2, pc3, pc4]

            # ---- all constants in one DMA ----
            cs = cp.tile([128, NCC], F32, tag="cpack")
            nc.gpsimd.dma_start(out=cs, in_=cpack)
            nth_sb = cs[:, _C_NTH:_C_NTH + 1]
            ch4_sb = cs[:, _C_CH4:_C_CH4 + 1]
            dt4_sb = cs[:, _C_DT4:_C_DT4 + 128]
            mag_sb = cs[0:17, _C_MAG:_C_MAG + 18]
            mbg_sb = cs[0:17, _C_MBG:_C_MBG + 18]
            ctg_sb = cs[0:17, _C_CTG:_C_CTG + 18]
            mt_sb = cs[0:17, _C_MT:_C_MT + 17]
            ca_sb = cp.tile([MK, 16], BF16, tag="ca")
            nc.gpsimd.dma_start(out=ca_sb, in_=coefa)

            # x broadcast tiles for stats: batch b on partitions 32b..32b+31
            x4 = cp.tile([128, P], F32, tag="x4")
            for b in range(BPC):
                xr = xs[b, :]
                nc.sync.dma_start(out=x4[32 * b:32 * (b + 1), :], in_=bass.AP(
                    tensor=xr.tensor, offset=xr.offset,
                    ap=[[0, 32]] + xr.ap))

            ones_bf = cp.tile([1, P], BF16, tag="ones_bf")
            nc.vector.memset(ones_bf, 1.0)
            f0t_0 = cp.tile([18, P], BF16, tag="f0t0")
            f0t_1 = cp.tile([18, P], BF16, tag="f0t1")
            f0t_2 = cp.tile([18, P], BF16, tag="f0t2")
            f0t_3 = cp.tile([18, P], BF16, tag="f0t3")
            f0ts = [f0t_0, f0t_1, f0t_2, f0t_3]
            for b in range(BPC):
                nc.gpsimd.dma_start(out=f0ts[b][16:17, :], in_=ones_bf)
                # x row of F0e (bf16) via SWDGE cast-DMA
                nc.gpsimd.dma_start(out=f0ts[b][17:18, :], in_=xs[b, :])

            # ---- prologue: stats for ALL batches ----
            e_t = cp.tile([128, P], F32, tag="et")
            nc.scalar.activation(e_t, x4, AF.Exp, scale=ch4_sb)
            den = cp.tile([128, 1], F32, tag="den")
            nc.vector.reduce_sum(den, e_t, axis=mybir.AxisListType.X)
            g_sb = cp.tile([128, 1], F32, tag="g")
            nc.scalar.activation(g_sb, den, AF.Ln)
            # two 64-partition group matmuls (matmul base must be 0/32/64)
            s_sbs, s2_sbs = [], []
            for grp in range(2):
                gs = slice(64 * grp, 64 * (grp + 1))
                s_ps = pss.tile([64, 1], F32, tag="small")
                nc.tensor.matmul(
                    s_ps, cs[gs, _C_DT4 + 64 * grp:_C_DT4 + 64 * (grp + 1)],
                    g_sb[gs, :], start=True, stop=True)
                s_sb = cp.tile([64, 1], F32, tag=f"ssb{grp}",
                               name=f"ssb{grp}")
                nc.vector.tensor_copy(s_sb, s_ps)
                s2_sb = cp.tile([64, 1], F32, tag=f"s2sb{grp}",
                                name=f"s2sb{grp}")
                nc.scalar.activation(s2_sb, s_ps, AF.Identity, scale=s_sb)
                s_sbs.append(s_sb)
                s2_sbs.append(s2_sb)

            zsbs = {}

            def front(b):
                f0t = f0ts[b]
                xb = wp.tile([128, P], F32, tag="xb")
                xr = xs[b, :]
                nc.sync.dma_start(out=xb, in_=bass.AP(
                    tensor=xr.tensor, offset=xr.offset,
                    ap=[[0, 128]] + xr.ap))

                # ---- spline features -> qv/qb ----
                rf = wp.tile([MK, P], BF16, tag="rf")
                nc.scalar.activation(rf, xb, AF.Relu, bias=nth_sb)
                big = psa.tile([18, P], F32, tag="big")
                for half in range(2):
                    nc.tensor.matmul(
                        big[0:16, half * 512:(half + 1) * 512], ca_sb,
                        rf[:, half * 512:(half + 1) * 512],
                        start=True, stop=True)

                # ---- per-batch N_ext chain (tiny) ----
                off = 32 * (b % 2)
                sl = slice(off, off + 32)
                csl = slice(32 * b, 32 * (b + 1))
                ab_ps = pss.tile([49, 1], F32, tag="small")
                nc.tensor.matmul(ab_ps, cs[sl, _C_RA4:_C_RA4 + 49],
                                 s_sbs[b // 2][sl, :],
                                 start=True, stop=False)
                nc.tensor.matmul(ab_ps, cs[sl, _C_RB4:_C_RB4 + 49],
                                 s2_sbs[b // 2][sl, :],
                                 start=False, stop=True)
                ab_sb = wp.tile([49, 1], F32, tag="absb")
                nc.vector.tensor_copy(ab_sb, ab_ps)
                tg_a = wp.tile([17, 18], F32, tag="tga")
                nc.scalar.activation(tg_a, mag_sb, AF.Identity,
                                     scale=ab_sb[0:17, 0:1])
                tg_b = wp.tile([17, 18], F32, tag="tgb")
                nc.scalar.activation(tg_b, mbg_sb, AF.Identity,
                                     scale=ab_sb[32:49, 0:1])
                pg_ps = pss.tile([17, 18], F32, tag="small")
                nc.tensor.matmul(pg_ps, mt_sb, tg_a, start=True, stop=False)
                nc.tensor.matmul(pg_ps, mt_sb, tg_b, start=False, stop=False)
                nc.tensor.matmul(pg_ps, mt_sb, ctg_sb, start=False, stop=True)
                pg_sb = wp.tile([17, 18], F32, tag="pgsb")
                nc.vector.tensor_copy(pg_sb, pg_ps)
                ne_ps = pss.tile([18, 18], F32, tag="small")
                nc.tensor.matmul(ne_ps, tg_a, pg_sb, start=True, stop=False)
                nc.tensor.matmul(ne_ps, tg_b, pg_sb, start=False, stop=False)
                nc.tensor.matmul(ne_ps, ctg_sb, pg_sb, start=False, stop=True)
                ne_sb = wp.tile([18, 18], BF16, tag="nesb")
                nc.vector.tensor_copy(ne_sb, ne_ps)

                # F0e rows 0:16 (cast f32 psum -> bf16)
                nc.vector.tensor_copy(f0t[0:16, :], big[0:16, :])

                # Z = N_ext^T @ F0e^T [18, 1024] (reuse 'big' psum banks)
                zps = psa.tile([18, P], F32, tag="big")
                for half in range(2):
                    nc.tensor.matmul(zps[:, half * 512:(half + 1) * 512],
                                     ne_sb,
                                     f0t[:, half * 512:(half + 1) * 512],
                                     start=True, stop=True)
                z_sb = wp.tile([18, P], BF16, tag="zsb")
                nc.vector.tensor_copy(z_sb, zps)
                zsbs[b] = z_sb

            def back(b):
                z_sb = zsbs.pop(b)
                f0t = f0ts[b]
                # coeffs rows; 16 matmuls into 5 rotating psum banks,
                # copies alternate ACT/DVE; two 2MB DMAs per batch
                st = sp.tile([128, 8 * P], F32, tag="st")
                ob = out[b]
                for rc in range(8):
                    for half in range(2):
                        i = rc * 2 + half
                        cc = ccp[i % 5].tile([128, 512], F32, tag="cc")
                        nc.tensor.matmul(
                            cc, z_sb[:, rc * 128:(rc + 1) * 128],
                            f0t[:, half * 512:(half + 1) * 512],
                            start=True, stop=True)
                        dst = st[:, i * 512:(i + 1) * 512]
                        if i % 2 == 0:
                            nc.scalar.activation(dst, cc, AF.Identity)
                        else:
                            nc.vector.tensor_copy(dst, cc)
                    if rc in (3, 7):
                        hb = rc // 4
                        nc.sync.dma_start(
                            out=bass.AP(
                                tensor=ob.tensor,
                                offset=ob.offset + hb * 4 * 128 * P,
                                ap=[[P, 128], [128 * P, 4], [1, P]]),
                            in_=st[:, hb * 4 * P:(hb + 1) * 4 * P])

            # software pipeline: batch b+1's front overlaps batch b's
            # final block + output DMA
            front(0)
            for b in range(BPC):
                if b + 1 < BPC:
                    front(b + 1)
                back(b)
    nc.compile()
    return nc


def _host_consts(x, w_q, b_q, w_k, b_k, w_v, b_v, w_mem, w_u, b_u, w_v2,
                 b_v2):
    A = (w_k.reshape(H, D) @ w_mem.T)                     # (H, MEM)
    Wd = np.zeros((HID, 16), np.float64)
    Gu = np.zeros((17, RANK), np.float64)
    Gv = np.zeros((17, RANK), np.float64)
    for h in range(H):
        sl = slice(h * D, (h + 1) * D)
        Wd[sl, 2 * h] = w_v[sl]
        Wd[sl, 2 * h + 1] = b_v[sl]
        Gu[2 * h] = w_u[:, sl] @ w_v[sl]
        Gu[2 * h + 1] = w_u[:, sl] @ b_v[sl]
        Gv[2 * h] = w_v2[:, sl] @ w_v[sl]
        Gv[2 * h + 1] = w_v2[:, sl] @ b_v[sl]
    Gu[16] = b_u
    Gv[16] = b_v2
    Mp = Gu @ Gv.T                                        # (17,17)

    # linear-spline fit of qv/qb over the realized x range
    xmin, xmax = float(x.min()) - 0.02, float(x.max()) + 0.02
    grid = np.linspace(xmin, xmax, 6001)
    u = grid[:, None] * w_q + b_q
    phi = np.minimum(np.exp(u), 1.0) + np.maximum(u, 0.0)
    targ = phi @ Wd                                       # (6001, 16)
    theta = np.linspace(xmin, xmax, MK)
    Afit = np.concatenate([np.maximum(grid[:, None] - theta, 0),
                           np.ones((len(grid), 1)), grid[:, None]], 1)
    AtA = Afit.T @ Afit
    lam = 1e-10 * np.trace(AtA) / Afit.shape[1]
    coef = np.linalg.solve(AtA + lam * np.eye(MK + 2), Afit.T @ targ)
    cA, c0, c1 = coef[:MK], coef[MK], coef[MK + 1]

    G = np.zeros((18, 17))
    G[:16, :16] = np.eye(16)
    G[16, 16] = 1.0
    G[16, :16] = c0
    G[17, :16] = c1
    mA = np.zeros((17, 17))
    mB = np.zeros((17, 17))
    cT = np.zeros((17, 17))
    for h in range(H):
        mA[2 * h, 2 * h] = 1.0
        mB[2 * h, 2 * h + 1] = 1.0
        mB[2 * h + 1, 2 * h] = 1.0
        cT[2 * h + 1, 2 * h + 1] = float(MEM)
    cT[16, 16] = 1.0

    # chebyshev nodes over range of A; Dmat = derivative-at-nodes matrix;
    # RA/RB fold cardinal interpolation + per-head mem reduction
    lo, hi = float(A.min()), float(A.max())
    kk = np.arange(KN)
    nodes = (lo + hi) / 2 + (hi - lo) / 2 * np.cos(np.pi * (kk + 0.5) / KN)
    from numpy.polynomial import chebyshev as C

    def t(a):
        return (2 * a - (lo + hi)) / (hi - lo)

    Vninv = np.linalg.inv(C.chebvander(t(nodes), KN - 1))
    Dmat = np.zeros((KN, KN))
    for j in range(KN):
        Dmat[:, j] = C.chebval(t(nodes), C.chebder(Vninv[:, j])) * 2 / (hi - lo)
    L = C.chebvander(t(A.ravel()), KN - 1) @ Vninv        # (H*MEM, KN)
    R = L.reshape(H, MEM, KN).sum(1).T                    # (KN, H)
    RA = np.zeros((KN, 49), np.float32)
    RB = np.zeros((KN, 49), np.float32)
    for h in range(H):
        RA[:, 32 + 2 * h] = R[:, h]
        RA[:, 32 + 2 * h + 1] = R[:, h]
        RB[:, 2 * h] = R[:, h]

    cpack = np.zeros((128, NCC), np.float32)
    cpack[:, _C_NTH] = -theta
    cpack[:, _C_CH4] = np.tile(nodes, BPC)
    for b in range(BPC):
        sl = slice(32 * b, 32 * (b + 1))
        cpack[sl, _C_DT4 + 32 * b:_C_DT4 + 32 * (b + 1)] = Dmat.T
        cpack[sl, _C_RA4:_C_RA4 + 49] = RA
        cpack[sl, _C_RB4:_C_RB4 + 49] = RB
    cpack[0:17, _C_MAG:_C_MAG + 18] = mA @ G.T
    cpack[0:17, _C_MBG:_C_MBG + 18] = mB @ G.T
    cpack[0:17, _C_CTG:_C_CTG + 18] = cT @ G.T
    cpack[0:17, _C_MT:_C_MT + 17] = Mp.T
    return {"cpack": cpack,
            "coefa": cA.astype(ml_dtypes.bfloat16)}


def kernel(**inputs):
    x = np.ascontiguousarray(inputs["x"], dtype=np.float32)
    consts = _host_consts(
        x.astype(np.float64),
        *(np.asarray(inputs[k], np.float64) for k in
          ["w_q", "b_q", "w_k", "b_k", "w_v", "b_v", "w_mem",
           "w_u", "b_u", "w_v2", "b_v2"]))
    if "nc" not in _CACHE:
        _CACHE["nc"] = _build()
    nc = _CACHE["nc"]
    in_maps = []
    for c in range(NCORES):
        in_maps.append({"xs": x[c * BPC:(c + 1) * BPC].copy(), **consts})
    res = bass_utils.run_bass_kernel_spmd(
        nc, in_maps, core_ids=list(range(NCORES)), trace=TRACE)
    _CACHE["last_res"] = res
    return np.concatenate([res.results[c]["out"] for c in range(NCORES)], 0)


# revision 24
# speedup vs baseline: 1.1082x; 1.1082x over previous
"""Trainium2 Bass kernel for nn_LinearCoeffGNN: coeffs = U @ Vp^T pipeline.

Exact factorization of the reference:  coeffs[b] = F0e @ N_ext @ F0e^T

  F0e = [qv_0 qb_0 .. qv_7 qb_7 | 1 | x]  (P x 18, bf16) where
  qv_h(x), qb_h(x) are scalar C1 functions of x (the Linear(1,hid) layers
  make everything rank-1 in x).  They are evaluated as a 128-knot linear
  spline: ONE Relu activation rfeat[j,p] = relu(x_p - theta_j) plus a
  [128,16] matmul; the const/linear spline terms fold into N_ext via
  G rows 16/17 (fit max err 8e-4 on range 31).

  softmax stats: s(A) = num/den are moment generating functions of x
  (entire in A), so only den at KN chebyshev nodes is needed; all four
  batches' stats run as ONE [128,P] Exp + ONE reduce + ONE Ln + ONE
  block-diagonal Dmat matmul in the prologue (derivative of the
  chebyshev interpolant), then per-head sums S1,S2 via [32,49] matmuls.

  N_ext = G T' Mp T' G^T built as TG^T (Mp TG), TG = T'G^T from two
  Identity(scale) activations + 6 tiny accumulating matmuls.

  The batch loop only uses Relu/Identity activations (one act-table
  set: no table reloads), all consts arrive in one packed DMA, and
  batch b+1's front is software-pipelined over batch b's final block.
Sharding: data-parallel over batch B=32 -> 4 batches per core on 8 cores.
"""
import numpy as np
import ml_dtypes

import concourse.bacc as bacc
import concourse.bass as bass
import concourse.mybir as mybir
import concourse.tile as tile
from concourse import bass_utils

B, P = 32, 1024
HID, H, D = 512, 8, 64
MEM, RANK = 64, 64
NCORES = 8
BPC = B // NCORES  # batches per core
KN = 32            # chebyshev nodes for the softmax-stats interpolation
MK = 128           # spline knots for qv/qb evaluation

F32 = mybir.dt.float32
BF16 = mybir.dt.bfloat16
AF = mybir.ActivationFunctionType
ALU = mybir.AluOpType

_CACHE = {}
TRACE = False

# packed const layout: columns of a [128, NCC] f32 tensor
_C_NTH = 0      # [128,1] -theta
_C_CH4 = 1      # [128,1] chebyshev nodes tiled x4
_C_DT4 = 2      # [128,128] block-diag Dmat^T x4
_C_RA4 = 130    # [128,49] RA tiled x4
_C_RB4 = 179    # [128,49] RB tiled x4
_C_MAG = 228    # [17,18] mA G^T
_C_MBG = 246    # [17,18] mB G^T
_C_CTG = 264    # [17,18] cT G^T
_C_MT = 282     # [17,17] Mp^T
_C_CA = 299     # [128,8] coefa bf16 viewed as f32
NCC = 307


def _build():
    nc = bacc.Bacc("TRN2", target_bir_lowering=False, debug=False,
                   num_devices=NCORES)
    xs = nc.dram_tensor("xs", [BPC, P], F32, kind="ExternalInput").ap()
    cpack = nc.dram_tensor("cpack", [128, NCC], F32, kind="ExternalInput").ap()
    coefa = nc.dram_tensor("coefa", [MK, 16], BF16, kind="ExternalInput").ap()
    out = nc.dram_tensor("out", [BPC, P, P], F32, kind="ExternalOutput").ap()

    with tile.TileContext(nc) as tc:
        with tc.tile_pool(name="consts", bufs=1) as cp, \
             tc.tile_pool(name="work", bufs=2) as wp, \
             tc.tile_pool(name="stage", bufs=2) as sp, \
             tc.tile_pool(name="ps_big", bufs=1, space="PSUM") as psa, \
             tc.tile_pool(name="ps_small", bufs=1, space="PSUM") as pss, \
             tc.tile_pool(name="ps_c0", bufs=1, space="PSUM") as pc0, \
             tc.tile_pool(name="ps_c1", bufs=1, space="PSUM") as pc1, \
             tc.tile_pool(name="ps_c2", bufs=1, space="PSUM") as pc2, \
             tc.tile_pool(name="ps_c3", bufs=1, space="PSUM") as pc3, \
             tc.tile_pool(name="ps_c4", bufs=1, space="PSUM") as pc4:
            ccp = [pc0, pc1, pc2, pc3, pc4]

            # ---- all constants in one DMA ----
            cs = cp.tile([128, NCC], F32, tag="cpack")
            nc.gpsimd.dma_start(out=cs, in_=cpack)
            nth_sb = cs[:, _C_NTH:_C_NTH + 1]
            ch4_sb = cs[:, _C_CH4:_C_CH4 + 1]
            dt4_sb = cs[:, _C_DT4:_C_DT4 + 128]
            mag_sb = cs[0:17, _C_MAG:_C_MAG + 18]
            mbg_sb = cs[0:17, _C_MBG:_C_MBG + 18]
            ctg_sb = cs[0:17, _C_CTG:_C_CTG + 18]
            mt_sb = cs[0:17, _C_MT:_C_MT + 17]
            ca_sb = cp.tile([MK, 16], BF16, tag="ca")
            nc.gpsimd.dma_start(out=ca_sb, in_=coefa)

            # x broadcast tile for stats: batch b on partitions 32b..32b+31
            x4 = cp.tile([128, P], F32, tag="x4")
            nc.sync.dma_start(out=x4, in_=bass.AP(
                tensor=xs.tensor, offset=xs.offset,
                ap=[[P, BPC], [0, 32], [1, P]]))
            # dummy activations: settle the act table (ln+exp share a set)
            dz = cp.tile([1, 1], F32, tag="dz")
            nc.vector.memset(dz, 1.0)
            dz2 = cp.tile([1, 1], F32, tag="dz2")
            nc.scalar.activation(dz2, dz, AF.Ln)
            nc.scalar.activation(dz2, dz, AF.Exp)

            ones_bf = cp.tile([1, P], BF16, tag="ones_bf")
            nc.vector.memset(ones_bf, 1.0)
            f0t_0 = cp.tile([18, P], BF16, tag="f0t0")
            f0t_1 = cp.tile([18, P], BF16, tag="f0t1")
            f0t_2 = cp.tile([18, P], BF16, tag="f0t2")
            f0t_3 = cp.tile([18, P], BF16, tag="f0t3")
            f0ts = [f0t_0, f0t_1, f0t_2, f0t_3]
            for b in range(BPC):
                nc.gpsimd.dma_start(out=f0ts[b][16:17, :], in_=ones_bf)
                # x row of F0e (bf16) via SWDGE cast-DMA
                nc.gpsimd.dma_start(out=f0ts[b][17:18, :], in_=xs[b, :])

            # ---- prologue: stats for ALL batches ----
            e_t = cp.tile([128, P], F32, tag="et")
            nc.scalar.activation(e_t, x4, AF.Exp, scale=ch4_sb)
            den = cp.tile([128, 1], F32, tag="den")
            nc.vector.reduce_sum(den, e_t, axis=mybir.AxisListType.X)
            g_sb = cp.tile([128, 1], F32, tag="g")
            nc.scalar.activation(g_sb, den, AF.Ln)
            # two 64-partition group matmuls (matmul base must be 0/32/64)
            s_sbs, s2_sbs = [], []
            for grp in range(2):
                gs = slice(64 * grp, 64 * (grp + 1))
                s_ps = pss.tile([64, 1], F32, tag="small")
                nc.tensor.matmul(
                    s_ps, cs[gs, _C_DT4 + 64 * grp:_C_DT4 + 64 * (grp + 1)],
                    g_sb[gs, :], start=True, stop=True)
                s_sb = cp.tile([64, 1], F32, tag=f"ssb{grp}",
                               name=f"ssb{grp}")
                nc.vector.tensor_copy(s_sb, s_ps)
                s2_sb = cp.tile([64, 1], F32, tag=f"s2sb{grp}",
                                name=f"s2sb{grp}")
                nc.scalar.activation(s2_sb, s_ps, AF.Identity, scale=s_sb)
                s_sbs.append(s_sb)
                s2_sbs.append(s2_sb)

            zsbs = {}

            def front(b):
                f0t = f0ts[b]
                xb = wp.tile([128, P], F32, tag="xb")
                xr = xs[b, :]
                nc.sync.dma_start(out=xb, in_=bass.AP(
                    tensor=xr.tensor, offset=xr.offset,
                    ap=[[0, 128]] + xr.ap))

                # ---- spline features -> qv/qb ----
                rf = wp.tile([MK, P], BF16, tag="rf")
                nc.scalar.activation(rf, xb, AF.Relu, bias=nth_sb)
                big = psa.tile([18, P], F32, tag="big")
                for half in range(2):
                    nc.tensor.matmul(
                        big[0:16, half * 512:(half + 1) * 512], ca_sb,
                        rf[:, half * 512:(half + 1) * 512],
                        start=True, stop=True)

                # ---- per-batch N_ext chain (tiny) ----
                off = 32 * (b % 2)
                sl = slice(off, off + 32)
                csl = slice(32 * b, 32 * (b + 1))
                ab_ps = pss.tile([49, 1], F32, tag="small")
                nc.tensor.matmul(ab_ps, cs[sl, _C_RA4:_C_RA4 + 49],
                                 s_sbs[b // 2][sl, :],
                                 start=True, stop=False)
                nc.tensor.matmul(ab_ps, cs[sl, _C_RB4:_C_RB4 + 49],
                                 s2_sbs[b // 2][sl, :],
                                 start=False, stop=True)
                ab_sb = wp.tile([49, 1], F32, tag="absb")
                nc.vector.tensor_copy(ab_sb, ab_ps)
                tg_a = wp.tile([17, 18], F32, tag="tga")
                nc.scalar.activation(tg_a, mag_sb, AF.Identity,
                                     scale=ab_sb[0:17, 0:1])
                tg_b = wp.tile([17, 18], F32, tag="tgb")
                nc.scalar.activation(tg_b, mbg_sb, AF.Identity,
                                     scale=ab_sb[32:49, 0:1])
                pg_ps = pss.tile([17, 18], F32, tag="small")
                nc.tensor.matmul(pg_ps, mt_sb, tg_a, start=True, stop=False)
                nc.tensor.matmul(pg_ps, mt_sb, tg_b, start=False, stop=False)
                nc.tensor.matmul(pg_ps, mt_sb, ctg_sb, start=False, stop=True)
                pg_sb = wp.tile([17, 18], F32, tag="pgsb")
                nc.vector.tensor_copy(pg_sb, pg_ps)
                ne_ps = pss.tile([18, 18], F32, tag="small")
                nc.tensor.matmul(ne_ps, tg_a, pg_sb, start=True, stop=False)
                nc.tensor.matmul(ne_ps, tg_b, pg_sb, start=False, stop=False)
                nc.tensor.matmul(ne_ps, ctg_sb, pg_sb, start=False, stop=True)
                ne_sb = wp.tile([18, 18], BF16, tag="nesb")
                nc.vector.tensor_copy(ne_sb, ne_ps)

                # F0e rows 0:16 (cast f32 psum -> bf16)
                nc.vector.tensor_copy(f0t[0:16, :], big[0:16, :])

                # Z = N_ext^T @ F0e^T [18, 1024] (reuse 'big' psum banks)
                zps = psa.tile([18, P], F32, tag="big")
                for half in range(2):
                    nc.tensor.matmul(zps[:, half * 512:(half + 1) * 512],
                                     ne_sb,
                                     f0t[:, half * 512:(half + 1) * 512],
                                     start=True, stop=True)
                z_sb = wp.tile([18, P], BF16, tag="zsb")
                nc.vector.tensor_copy(z_sb, zps)
                zsbs[b] = z_sb

            def back(b):
                z_sb = zsbs.pop(b)
                f0t = f0ts[b]
                # coeffs rows; 16 matmuls into 5 rotating psum banks,
                # copies alternate ACT/DVE; two 2MB DMAs per batch
                st = sp.tile([128, 8 * P], F32, tag="st")
                ob = out[b]
                for rc in range(8):
                    for half in range(2):
                        i = rc * 2 + half
                        cc = ccp[i % 5].tile([128, 512], F32, tag="cc")
                        nc.tensor.matmul(
                            cc, z_sb[:, rc * 128:(rc + 1) * 128],
                            f0t[:, half * 512:(half + 1) * 512],
                            start=True, stop=True)
                        dst = st[:, i * 512:(i + 1) * 512]
                        if i % 2 == 0:
                            nc.scalar.activation(dst, cc, AF.Identity)
                        else:
                            nc.vector.tensor_copy(dst, cc)
                    if rc in (3, 7):
                        hb = rc // 4
                        nc.sync.dma_start(
                            out=bass.AP(
                                tensor=ob.tensor,
                                offset=ob.offset + hb * 4 * 128 * P,
                                ap=[[P, 128], [128 * P, 4], [1, P]]),
                            in_=st[:, hb * 4 * P:(hb + 1) * 4 * P])

            # software pipeline: batch b+1's front overlaps batch b's
            # final block + output DMA
            front(0)
            for b in range(BPC):
                if b + 1 < BPC:
                    front(b + 1)
                back(b)
    nc.compile()
    return nc


def _host_consts(x, w_q, b_q, w_k, b_k, w_v, b_v, w_mem, w_u, b_u, w_v2,
                 b_v2):
    A = (w_k.reshape(H, D) @ w_mem.T)                     # (H, MEM)
    Wd = np.zeros((HID, 16), np.float64)
    Gu = np.zeros((17, RANK), np.float64)
    Gv = np.zeros((17, RANK), np.float64)
    for h in range(H):
        sl = slice(h * D, (h + 1) * D)
        Wd[sl, 2 * h] = w_v[sl]
        Wd[sl, 2 * h + 1] = b_v[sl]
        Gu[2 * h] = w_u[:, sl] @ w_v[sl]
        Gu[2 * h + 1] = w_u[:, sl] @ b_v[sl]
        Gv[2 * h] = w_v2[:, sl] @ w_v[sl]
        Gv[2 * h + 1] = w_v2[:, sl] @ b_v[sl]
    Gu[16] = b_u
    Gv[16] = b_v2
    Mp = Gu @ Gv.T                                        # (17,17)

    # linear-spline fit of qv/qb over the realized x range
    xmin, xmax = float(x.min()) - 0.02, float(x.max()) + 0.02
    grid = np.linspace(xmin, xmax, 6001)
    u = grid[:, None] * w_q + b_q
    phi = np.minimum(np.exp(u), 1.0) + np.maximum(u, 0.0)
    targ = phi @ Wd                                       # (6001, 16)
    theta = np.linspace(xmin, xmax, MK)
    Afit = np.concatenate([np.maximum(grid[:, None] - theta, 0),
                           np.ones((len(grid), 1)), grid[:, None]], 1)
    AtA = Afit.T @ Afit
    lam = 1e-10 * np.trace(AtA) / Afit.shape[1]
    coef = np.linalg.solve(AtA + lam * np.eye(MK + 2), Afit.T @ targ)
    cA, c0, c1 = coef[:MK], coef[MK], coef[MK + 1]

    G = np.zeros((18, 17))
    G[:16, :16] = np.eye(16)
    G[16, 16] = 1.0
    G[16, :16] = c0
    G[17, :16] = c1
    mA = np.zeros((17, 17))
    mB = np.zeros((17, 17))
    cT = np.zeros((17, 17))
    for h in range(H):
        mA[2 * h, 2 * h] = 1.0
        mB[2 * h, 2 * h + 1] = 1.0
        mB[2 * h + 1, 2 * h] = 1.0
        cT[2 * h + 1, 2 * h + 1] = float(MEM)
    cT[16, 16] = 1.0

    # chebyshev nodes over range of A; Dmat = derivative-at-nodes matrix;
    # RA/RB fold cardinal interpolation + per-head mem reduction
    lo, hi = float(A.min()), float(A.max())
    kk = np.arange(KN)
    nodes = (lo + hi) / 2 + (hi - lo) / 2 * np.cos(np.pi * (kk + 0.5) / KN)
    from numpy.polynomial import chebyshev as C

    def t(a):
        return (2 * a - (lo + hi)) / (hi - lo)

    Vninv = np.linalg.inv(C.chebvander(t(nodes), KN - 1))
    Dmat = np.zeros((KN, KN))
    for j in range(KN):
        Dmat[:, j] = C.chebval(t(nodes), C.chebder(Vninv[:, j])) * 2 / (hi - lo)
    L = C.chebvander(t(A.ravel()), KN - 1) @ Vninv        # (H*MEM, KN)
    R = L.reshape(H, MEM, KN).sum(1).T                    # (KN, H)
    RA = np.zeros((KN, 49), np.float32)
    RB = np.zeros((KN, 49), np.float32)
    for h in range(H):
        RA[:, 32 + 2 * h] = R[:, h]
        RA[:, 32 + 2 * h + 1] = R[:, h]
        RB[:, 2 * h] = R[:, h]

    cpack = np.zeros((128, NCC), np.float32)
    cpack[:, _C_NTH] = -theta
    cpack[:, _C_CH4] = np.tile(nodes, BPC)
    for b in range(BPC):
        sl = slice(32 * b, 32 * (b + 1))
        cpack[sl, _C_DT4 + 32 * b:_C_DT4 + 32 * (b + 1)] = Dmat.T
        cpack[sl, _C_RA4:_C_RA4 + 49] = RA
        cpack[sl, _C_RB4:_C_RB4 + 49] = RB
    cpack[0:17, _C_MAG:_C_MAG + 18] = mA @ G.T
    cpack[0:17, _C_MBG:_C_MBG + 18] = mB @ G.T
    cpack[0:17, _C_CTG:_C_CTG + 18] = cT @ G.T
    cpack[0:17, _C_MT:_C_MT + 17] = Mp.T
    return {"cpack": cpack,
            "coefa": cA.astype(ml_dtypes.bfloat16)}


def kernel(**inputs):
    x = np.ascontiguousarray(inputs["x"], dtype=np.float32)
    consts = _host_consts(
        x.astype(np.float64),
        *(np.asarray(inputs[k], np.float64) for k in
          ["w_q", "b_q", "w_k", "b_k", "w_v", "b_v", "w_mem",
           "w_u", "b_u", "w_v2", "b_v2"]))
    if "nc" not in _CACHE:
        _CACHE["nc"] = _build()
    nc = _CACHE["nc"]
    in_maps = []
    for c in range(NCORES):
        in_maps.append({"xs": x[c * BPC:(c + 1) * BPC].copy(), **consts})
    res = bass_utils.run_bass_kernel_spmd(
        nc, in_maps, core_ids=list(range(NCORES)), trace=TRACE)
    _CACHE["last_res"] = res
    return np.concatenate([res.results[c]["out"] for c in range(NCORES)], 0)
